# revision 1
# baseline (speedup 1.0000x reference)
"""Trainium2 Bass kernel: two-layer LIF spiking network scan.

Model (per timestep t, batch row b):
    h1 = x_t @ W1.T + b1            # [B, 32]
    v1 = v1 + (h1 - v1)/2           # tau = 2
    s1 = (v1 >= 1);  v1 *= (1-s1)   # hard reset
    h2 = s1 @ W2.T + b2             # [B, 1]
    v2 = v2 + (h2 - v2)/2
    s2 = (v2 >= 1);  v2 *= (1-s2)
    out = sum of s2 over t in [T - T//4, T)

Kernel strategy (pure data parallel over batch, 8 cores x 512 rows;
batch rows on the 128 SBUF partitions, 4 groups of 128 rows in the
free dimension):

  - The input current c_t = x_t @ (W1.T/2) is precomputed on the PE
    (tensor) engine in 32-step chunks: stationary operand = transposed
    x tiles (prepared host-side, K=64 rows = 32 steps x 2 inputs, two
    partition phases at bases 0/64), moving operand = a block-diagonal
    W1/2 pattern covering 16 of the 32 hidden units (N=512), both
    bitcast to float32r (exact fp32 bits, 1 cycle/row at N=512).  The
    scalar (Act) engine copies each PSUM tile into an SBUF ring.
  - The vector engine runs the serial t-loop with 2 fused custom DVE
    ops per step; layer-1 state is the PRE-reset potential u (spikes
    are just u >= 1):
      LIF1   u' = (u < 1) ? 0.5*u + c : c         (decay + hard reset)
      SDS    prefix-sum along free of (u' >= 1)*(W2h/2) -> per-group
             layer-2 input via a strided difference of the prefix sums
  - The gpsimd (Pool) engine consumes the scan ring in 8-step blocks:
    one batched strided-difference op per block, then the tiny per-step
    layer-2 LIF chain, with the decision-window spike counting batched
    per block.
"""

import numpy as np

B, T, I, H, O = 4096, 4096, 2, 32, 1
N_CORES = 8
B_CORE = B // N_CORES          # 512
G = B_CORE // 128              # 4 groups
FW = G * H                     # 128 fused free width
TCH = 32                       # timesteps per PE chunk (K = TCH*I = 64)
HH = H // 2                    # hidden units per matmul (N = TCH*HH = 512)
BLK = 8                        # timesteps per gpsimd block
NSLOT = 2 * BLK                # scan ring depth (2 blocks)
SW = FW + 4                    # scan slot width (1 lo-zero col + 128 + pad)

_cache = {}


# ----------------------------------------------------------------- custom ops
def _register_custom_ops():
    """Register our custom DVE ops in the process-global registry (idempotent)."""
    import concourse.dve_ops as dve_ops_mod
    from concourse.dve_ops import DveOp
    from concourse.dve_spec import (
        Spec, Src0, Src1, C0, C1, C2, Zero, One,
        select, eq, lower, AluOp, scan, _has_src1,
    )
    from concourse.dve_uop import DveOpSpec

    if "ANT_SNN_FMA2" in dve_ops_mod._SUB_OPCODE_FOR_NAME:
        return

    def _ref_fma2(in0, in1, s0, s1, imm2):
        return (in0 * s0 + in1 * s1).astype(np.float32)

    def _ref_lif1(in0, in1, s0, s1, imm2):
        # state is the pre-reset potential u: u' = (u<1) ? 0.5u + c : c
        return np.where(
            in0 < 1.0, (in0 * np.float32(0.5)) + in1, in1
        ).astype(np.float32)

    def _ref_sds(in0, in1, s0, s1, imm2):
        # prefix sums of (u >= 1) * w2h along the free dim
        contrib = np.where(in0 < 1.0, np.float32(0.0), in1)
        return np.cumsum(contrib.astype(np.float32), axis=-1, dtype=np.float32)

    specs = [
        ("ANT_SNN_FMA2", Spec(body=Src0 * C0 + Src1 * C1, reference=_ref_fma2)),
        (
            "ANT_SNN_LIF1",
            Spec(
                body=select(Src0 < One, Src0 * C0 + Src1, Src1),
                reference=_ref_lif1,
            ),
        ),
        (
            "ANT_SNN_SDS",
            Spec(
                body=scan(AluOp.ADD, select(Src0 < One, Zero, Src1)),
                reference=_ref_sds,
            ),
        ),
    ]

    ops = {}
    for name, spec in specs:
        row = 1 + len(dve_ops_mod.OPS)
        sha = {}
        for ver in ("v3", "v4"):
            try:
                s = DveOpSpec(
                    name=name,
                    opcode=row,
                    uops=lower(spec, ver=ver),
                    rd1_en=_has_src1(spec),
                )
                sha[ver] = s.sha(ver)
            except Exception:
                pass
        op = DveOp(name, spec, subdim=False, uops_sha=sha)
        dve_ops_mod.OPS.append(op)
        dve_ops_mod.CUSTOM_DVE_SPECS[name] = spec
        dve_ops_mod._SUB_OPCODE_FOR_NAME[name] = row
        ops[name] = op
    return ops


def _get_ops():
    import concourse.dve_ops as dve_ops_mod

    _register_custom_ops()
    by_name = {op.name: op for op in dve_ops_mod.OPS}
    return (
        by_name["ANT_SNN_FMA2"],
        by_name["ANT_SNN_LIF1"],
        by_name["ANT_SNN_SDS"],
    )


# ----------------------------------------------------------------- bass build
def build_nc_exact(t_steps=T, decision_start=None, has_b1=False, has_b2=False):
    """Build the per-core Bass program (SPMD; all cores run the same NEFF)."""
    import concourse.bass as bass
    import concourse.mybir as mybir

    _, OP_LIF1, OP_SDS = _get_ops()
    A = mybir.AluOpType
    f32 = mybir.dt.float32
    f32r = mybir.dt.float32r

    if decision_start is None:
        decision_start = max(t_steps - t_steps // 4, t_steps // 2)

    NCH = t_steps // TCH          # matmul chunks
    NCQ = NCH // 2                # chunk pairs (2 partition phases)
    NBLK = t_steps // BLK         # gpsimd blocks
    WINBLK = decision_start // BLK
    assert t_steps % (2 * TCH) == 0, "t_steps must be divisible by 64"
    assert decision_start % BLK == 0, "decision window must align to blocks"
    NW = (NCQ + 7) // 8           # x DMA waves (8 chunk pairs = 16 chunks each)
    MMC = 2 * G                   # matmuls (= Act copies) per chunk

    # Same-engine RAW hazards are safe on HW (per-op DVE pipeline drain);
    # the CoreSim race detector would flag them, so turn it off.
    nc = bass.Bass(detect_race_conditions=False)

    # xtb: transposed x tiles. partition p = 64*(chunk%2) + 2*t' + i,
    # free = (chunk//2, group, batch-row-in-group).
    xtb = nc.declare_dram_parameter("xtb", [128, NCQ * G * 128], f32r, isOutput=False)
    # wrhsb: block-diagonal W1/2 pattern, replicated on both 64-partition
    # phases: wrhsb[64*j + 2*t' + i, 512*half + 16*t'' + hh] =
    #   (t'==t'') * W1[16*half + hh, i]/2
    wrhsb = nc.declare_dram_parameter("wrhsb", [128, 2 * TCH * HH], f32r, isOutput=False)
    w2hb = nc.declare_dram_parameter("w2hb", [128, FW], f32, isOutput=False)
    k2b = nc.declare_dram_parameter("k2b", [128, 1], f32, isOutput=False)
    b1hb = nc.declare_dram_parameter("b1hb", [128, TCH * H], f32, isOutput=False)
    out = nc.declare_dram_parameter("out", [128, G], f32, isOutput=True)

    xsb = nc.alloc_sbuf_tensor("xsb", [128, NCQ, G, 128], f32r).ap()
    wsb = nc.alloc_sbuf_tensor("wsb", [128, 2 * TCH * HH], f32r).ap()
    w2h = nc.alloc_sbuf_tensor("w2h", [128, FW], f32).ap()
    k2 = nc.alloc_sbuf_tensor("k2", [128, 1], f32).ap()
    b1h = nc.alloc_sbuf_tensor("b1h", [128, TCH * H], f32).ap()
    # c ring: [phase, group, step-in-chunk, h]; also viewed per half for the
    # Act copies as [:, ph, g, :, half*16:(half+1)*16]
    cb = nc.alloc_sbuf_tensor("cb", [128, 2, G, TCH, H], f32).ap()
    S0 = nc.alloc_sbuf_tensor("S0", [128, FW], f32).ap()
    S1 = nc.alloc_sbuf_tensor("S1", [128, FW], f32).ap()
    scanring = nc.alloc_sbuf_tensor("scanring", [128, NSLOT, SW], f32).ap()
    rbuf = nc.alloc_sbuf_tensor("rbuf", [128, BLK, G], f32).ap()
    u2r = nc.alloc_sbuf_tensor("u2r", [128, BLK, G], f32).ap()
    s32 = nc.alloc_sbuf_tensor("s32", [128, BLK, G], f32).ap()
    q2 = nc.alloc_sbuf_tensor("q2", [128, G], f32).ap()
    y2 = nc.alloc_sbuf_tensor("y2", [128, G], f32).ap()
    acc32 = nc.alloc_sbuf_tensor("acc32", [128, BLK, G], f32).ap()
    S_pp = [S0, S1]

    # PSUM: 4 groups x 2 halves of [128, 512] = all 8 banks (single buffer;
    # PE->Act->DVE pipelining is per (group, half) tile)
    pb = [
        [nc.alloc_psum_tensor(f"pb{g}_{hf}", [128, TCH * HH], f32).ap()
         for hf in range(2)]
        for g in range(G)
    ]

    with (
        nc.semaphore("dma_w") as dma_w,
        nc.semaphore("dma_x") as dma_x,
        nc.semaphore("pe_done") as pe_done,
        nc.semaphore("act_done") as act_done,
        nc.semaphore("v_done") as v_done,
        nc.semaphore("d2g") as d2g,
        nc.semaphore("g2d") as g2d,
        nc.semaphore("g_done") as g_done,
        nc.Block() as block,
    ):
        @block.sync
        def _(sync):
            sync.dma_start(out=wsb[:], in_=wrhsb[:]).then_inc(dma_w, 16)
            sync.dma_start(out=w2h[:], in_=w2hb[:]).then_inc(dma_w, 16)
            sync.dma_start(out=k2[:], in_=k2b[:]).then_inc(dma_w, 16)
            sync.dma_start(out=b1h[:], in_=b1hb[:]).then_inc(dma_w, 16)
            # x waves issued serially so the completion count is monotone
            for w in range(NW):
                q0, q1 = 8 * w, min(8 * w + 8, NCQ)
                sync.dma_start(
                    out=xsb[:, q0:q1, :, :],
                    in_=xtb[:, q0 * G * 128 : q1 * G * 128],
                ).then_inc(dma_x, 16)
                sync.wait_ge(dma_x, 16 * (w + 1))
            sync.wait_ge(g_done, 1)
            sync.dma_start(out=out[:, :], in_=acc32[:, 0, :]).then_inc(dma_w, 16)
            sync.wait_ge(dma_w, 16 * 5)

        @block.tensor
        def _(pe):
            pe.wait_ge(dma_w, 16 * 4)  # wsb (inc order across DMAs varies)
            for c in range(NCH):
                jc, cq = c % 2, c // 2
                if c % 16 == 0:
                    pe.wait_ge(dma_x, 16 * (c // 16 + 1))
                p0 = 64 * jc
                for g in range(G):
                    for hf in range(2):
                        if c >= 1:
                            # Act has drained this PSUM tile (chunk c-1)
                            pe.wait_ge(
                                act_done, MMC * (c - 1) + 2 * g + hf + 1
                            )
                        pe.matmul(
                            out=pb[g][hf][:],
                            lhsT=xsb[p0:p0 + 64, cq, g, :],
                            rhs=wsb[p0:p0 + 64, hf * 512:(hf + 1) * 512],
                            start=True,
                            stop=True,
                        ).then_inc(pe_done, 1)

        @block.scalar
        def _(act):
            for c in range(NCH):
                ph = c % 2
                if c >= 2:
                    # vector has finished reading this phase of the cb ring
                    act.wait_ge(v_done, c - 1)
                for g in range(G):
                    for hf in range(2):
                        act.wait_ge(pe_done, MMC * c + 2 * g + hf + 1)
                        act.copy(
                            out=cb[:, ph, g, :, hf * HH:(hf + 1) * HH],
                            in_=pb[g][hf][:],
                        ).then_inc(act_done, 1)

        @block.vector
        def _(vector):
            vector.memset(S_pp[0][:], 0.0)
            vector.memset(scanring[:], 0.0)
            vector.memset(y2[:], 0.0)
            vector.memset(acc32[:], 0.0)
            vector.wait_ge(dma_w, 16 * 4)  # weight tiles
            for t in range(t_steps):
                c, r, k = t // TCH, t % TCH, t // BLK
                if r == 0:
                    vector.wait_ge(act_done, MMC * (c + 1))
                    if has_b1:
                        for g in range(G):
                            vector.tensor_tensor(
                                out=cb[:, c % 2, g, :, :],
                                in0=cb[:, c % 2, g, :, :],
                                in1=b1h[:],
                                op=A.add,
                            )
                if t % BLK == 0 and k >= 2:
                    vector.wait_ge(g2d, k - 1)
                src = S_pp[t % 2]
                dst = S_pp[1 - t % 2]
                ins1 = vector._custom_dve(
                    OP_LIF1,
                    out=dst[:],
                    in0=src[:],
                    in1=cb[:, c % 2, :, r, :],
                    s0=0.5,
                )
                if r == TCH - 1:
                    ins1.then_inc(v_done, 1)
                ins2 = vector._custom_dve(
                    OP_SDS,
                    out=scanring[:, t % NSLOT, 1 : FW + 1],
                    in0=dst[:],
                    in1=w2h[:],
                )
                if t % BLK == BLK - 1:
                    ins2.then_inc(d2g, 1)

        @block.gpsimd
        def _(gpsimd):
            # Pool-legal ops only: tensor_scalar (incl. dual/compare) and
            # tensor_tensor add/mult/subtract.
            for k in range(NBLK):
                s0 = BLK * (k % 2)
                gpsimd.wait_ge(d2g, k + 1)
                # r_t for the whole block: strided differences of prefix sums
                hi = scanring[:, s0 : s0 + BLK, H : FW + 1 : H]
                lo = scanring[:, s0 : s0 + BLK, 0 : FW : H]
                gpsimd.tensor_tensor(out=rbuf[:], in0=hi, in1=lo, op=A.subtract)
                if has_b2:
                    gpsimd.tensor_scalar(rbuf[:], rbuf[:], k2[:], None, A.add)
                for j in range(BLK):
                    gpsimd.tensor_tensor(
                        out=u2r[:, j, :], in0=rbuf[:, j, :], in1=y2[:], op=A.add
                    )
                    # q2 = (u2 < 1) * 0.5  -> y2 = u2 * q2
                    gpsimd.tensor_scalar(
                        q2[:], u2r[:, j, :], 1.0, 0.5, A.is_lt, A.mult
                    )
                    ins = gpsimd.tensor_tensor(
                        out=y2[:], in0=u2r[:, j, :], in1=q2[:], op=A.mult
                    )
                if k >= WINBLK:
                    gpsimd.tensor_scalar(s32[:], u2r[:], 1.0, None, A.is_ge)
                    ins = gpsimd.tensor_tensor(
                        out=acc32[:], in0=acc32[:], in1=s32[:], op=A.add
                    )
                ins.then_inc(g2d, 1)
            # fold the 8 per-block-slot accumulators into slot 0
            h = BLK
            while h > 1:
                h //= 2
                gpsimd.tensor_tensor(
                    out=acc32[:, 0:h, :],
                    in0=acc32[:, 0:h, :],
                    in1=acc32[:, h : 2 * h, :],
                    op=A.add,
                )
            gpsimd.tensor_scalar(
                acc32[:, 0, :], acc32[:, 0, :], 1.0, None, A.mult
            ).then_inc(g_done, 1)

    # Populate .instr bytes for InstISA subclasses (custom DVE ops). Raw
    # Bass skips this pass; without it walrus fails with "ISA wrong length".
    mybir.codegen_inst_isa_subclasses(nc)
    return nc


def build_nc_fast(t_steps=T, decision_start=None, has_b1=False, has_b2=False):
    """Fast per-core Bass program.

    Differences vs build_nc_exact:
      - LIF1/SDS fused over 32-step pages: one LIF instruction both reads
        and writes the u-trajectory buffer offset by one 128-col step slot
        (the DVE element stream is sequential, and the 128-column spacing
        exceeds the SBUF write latency, so the within-instruction RAW is
        safe); one SDS instruction scans the page group-major, so
        consecutive strided taps of the prefix differ by exactly r_t.
      - Layer 2 is computed WITHOUT the hard reset as a plain affine
        recurrence (tensor_tensor_scan), which is exact whenever the
        no-reset potential stays < 1.  A second output `flg` counts
        timesteps with u2 >= 0.999 anywhere in [0, T); if nonzero the
        caller must fall back to the exact kernel (for the target model
        u2 tops out ~0.35, so the fast path is the one that runs).
    """
    import concourse.bass as bass
    import concourse.mybir as mybir

    _, OP_LIF1, OP_SDS = _get_ops()
    A = mybir.AluOpType
    f32 = mybir.dt.float32
    f32r = mybir.dt.float32r

    if decision_start is None:
        decision_start = max(t_steps - t_steps // 4, t_steps // 2)

    NB = t_steps // TCH           # blocks == matmul chunks (32 steps)
    NCQ = NB // 2                 # chunk pairs (2 partition phases)
    WINBLK = decision_start // TCH
    assert t_steps % (2 * TCH) == 0, "t_steps must be divisible by 64"
    assert decision_start % TCH == 0, "decision window must align to chunks"
    NW = (NCQ + 7) // 8           # x DMA waves (8 chunk pairs = 16 chunks)
    MMC = 2 * G                   # matmuls (= Act copies) per chunk
    PW = TCH * FW                 # 4096 elements per page

    nc = bass.Bass(detect_race_conditions=False)

    xtb = nc.declare_dram_parameter("xtb", [128, NCQ * G * 128], f32r, isOutput=False)
    wrhsb = nc.declare_dram_parameter("wrhsb", [128, 2 * TCH * HH], f32r, isOutput=False)
    w2hb = nc.declare_dram_parameter("w2hb", [128, FW], f32, isOutput=False)
    k2b = nc.declare_dram_parameter("k2b", [128, 1], f32, isOutput=False)
    b1hb = nc.declare_dram_parameter("b1hb", [128, FW], f32, isOutput=False)
    out = nc.declare_dram_parameter("out", [128, G], f32, isOutput=True)
    flg = nc.declare_dram_parameter("flg", [128, G], f32, isOutput=True)

    xsb = nc.alloc_sbuf_tensor("xsb", [128, NCQ, G, 128], f32r).ap()
    wsb = nc.alloc_sbuf_tensor("wsb", [128, 2 * TCH * HH], f32r).ap()
    w2s = nc.alloc_sbuf_tensor("w2s", [128, 1, H], f32).ap()
    k2 = nc.alloc_sbuf_tensor("k2", [128, 1], f32).ap()
    b1g = nc.alloc_sbuf_tensor("b1g", [128, 1, FW], f32).ap()
    # c ring, (t', g, h) layout so a whole chunk is LIF-stream-contiguous
    cb = nc.alloc_sbuf_tensor("cb", [128, 2, TCH, G, H], f32).ap()
    # u trajectory page: slot 0 = carry (u of the last step of the previous
    # page), slots 1..32 = this page's 32 steps
    U = nc.alloc_sbuf_tensor("U", [128, TCH + 1, G, H], f32).ap()
    scanout = nc.alloc_sbuf_tensor("scanout", [128, 1 + PW], f32).ap()
    rbuf = nc.alloc_sbuf_tensor("rbuf", [128, 2, FW], f32).ap()
    u2b = nc.alloc_sbuf_tensor("u2b", [128, 2, G, TCH], f32).ap()
    halfc = nc.alloc_sbuf_tensor("halfc", [128, TCH], f32).ap()
    sflag = nc.alloc_sbuf_tensor("sflag", [128, FW], f32).ap()
    swin = nc.alloc_sbuf_tensor("swin", [128, FW], f32).ap()
    accw = nc.alloc_sbuf_tensor("accw", [128, G, TCH], f32).ap()
    accf = nc.alloc_sbuf_tensor("accf", [128, G, TCH], f32).ap()
    outb = nc.alloc_sbuf_tensor("outb", [128, G], f32).ap()
    flgb = nc.alloc_sbuf_tensor("flgb", [128, G], f32).ap()

    pb = [
        [nc.alloc_psum_tensor(f"pb{g}_{hf}", [128, TCH * HH], f32).ap()
         for hf in range(2)]
        for g in range(G)
    ]

    u_page = U[:, 1 : TCH + 1, :, :].opt()               # (t', g, h) flat
    u_prev = U[:, 0:TCH, :, :].opt()
    w2bc = w2s.broadcast_to([128, TCH * G, H])            # stride-0 (t', g)
    b1bc = b1g.broadcast_to([128, TCH, FW])               # stride-0 t'

    with (
        nc.semaphore("dma_w") as dma_w,
        nc.semaphore("dma_x") as dma_x,
        nc.semaphore("pe_done") as pe_done,
        nc.semaphore("act_done") as act_done,
        nc.semaphore("v_done") as v_done,
        nc.semaphore("d2g") as d2g,
        nc.semaphore("p2v") as p2v,
        nc.semaphore("v2p") as v2p,
        nc.semaphore("g_done") as g_done,
        nc.Block() as block,
    ):
        @block.sync
        def _(sync):
            sync.dma_start(out=wsb[:], in_=wrhsb[:]).then_inc(dma_w, 16)
            sync.dma_start(out=w2s[:, 0, :], in_=w2hb[:, 0:H]).then_inc(dma_w, 16)
            sync.dma_start(out=k2[:], in_=k2b[:]).then_inc(dma_w, 16)
            sync.dma_start(out=b1g[:, 0, :], in_=b1hb[:]).then_inc(dma_w, 16)
            for w in range(NW):
                q0, q1 = 8 * w, min(8 * w + 8, NCQ)
                sync.dma_start(
                    out=xsb[:, q0:q1, :, :],
                    in_=xtb[:, q0 * G * 128 : q1 * G * 128],
                ).then_inc(dma_x, 16)
                sync.wait_ge(dma_x, 16 * (w + 1))
            sync.wait_ge(g_done, 1)
            sync.dma_start(out=out[:, :], in_=outb[:]).then_inc(dma_w, 16)
            sync.dma_start(out=flg[:, :], in_=flgb[:]).then_inc(dma_w, 16)
            sync.wait_ge(dma_w, 16 * 6)

        @block.tensor
        def _(pe):
            pe.wait_ge(dma_w, 16 * 4)
            for c in range(NB):
                jc, cq = c % 2, c // 2
                if c % 16 == 0:
                    pe.wait_ge(dma_x, 16 * (c // 16 + 1))
                p0 = 64 * jc
                for g in range(G):
                    for hf in range(2):
                        if c >= 1:
                            pe.wait_ge(
                                act_done, MMC * (c - 1) + 2 * g + hf + 1
                            )
                        pe.matmul(
                            out=pb[g][hf][:],
                            lhsT=xsb[p0:p0 + 64, cq, g, :],
                            rhs=wsb[p0:p0 + 64, hf * 512:(hf + 1) * 512],
                            start=True,
                            stop=True,
                        ).then_inc(pe_done, 1)

        @block.scalar
        def _(act):
            for c in range(NB):
                ph = c % 2
                if c >= 2:
                    act.wait_ge(v_done, c - 1)
                for g in range(G):
                    for hf in range(2):
                        act.wait_ge(pe_done, MMC * c + 2 * g + hf + 1)
                        act.copy(
                            out=cb[:, ph, :, g, hf * HH:(hf + 1) * HH],
                            in_=pb[g][hf][:],
                        ).then_inc(act_done, 1)

        @block.vector
        def _(vector):
            vector.memset(U[:], 0.0)
            vector.memset(scanout[:], 0.0)
            vector.memset(u2b[:], 0.0)
            vector.memset(accw[:], 0.0)
            vector.memset(accf[:], 0.0)
            vector.memset(halfc[:], 0.5)
            vector.wait_ge(dma_w, 16 * 4)
            for b in range(NB + 1):
                if b < NB:
                    vector.wait_ge(act_done, MMC * (b + 1))
                    if has_b1:
                        vector.tensor_tensor(
                            out=cb[:, b % 2, :, :, :].opt(),
                            in0=cb[:, b % 2, :, :, :].opt(),
                            in1=b1bc,
                            op=A.add,
                        )
                    # carry: u of previous page's last step -> slot 0
                    vector.tensor_scalar(
                        U[:, 0, :, :], U[:, TCH, :, :], 1.0, None, A.mult
                    )
                    ins1 = vector._custom_dve(
                        OP_LIF1,
                        out=u_page,
                        in0=u_prev,
                        in1=cb[:, b % 2, :, :, :].opt(),
                        s0=0.5,
                    )
                    ins1.then_inc(v_done, 1)
                    if b >= 1:
                        vector.wait_ge(p2v, b)  # pool drained scanout(b-1)
                    vector._custom_dve(
                        OP_SDS,
                        out=scanout[:, 1 : 1 + PW],
                        in0=u_page,
                        in1=w2bc,
                    ).then_inc(d2g, 1)
                if b >= 1:
                    bb = b - 1
                    pbi = bb % 2
                    if b == NB:
                        vector.wait_ge(p2v, NB)  # rbuf(NB-1) ready
                    ins = None
                    for g in range(G):
                        ins = vector.tensor_tensor_scan(
                            out=u2b[:, pbi, g, :],
                            data0=halfc[:],
                            data1=rbuf[:, pbi, g::G],
                            initial=u2b[:, 1 - pbi, g, TCH - 1 : TCH],
                            op0=A.mult,
                            op1=A.add,
                        )
                    ins.then_inc(v2p, 1)

        @block.gpsimd
        def _(gpsimd):
            for k in range(NB + 1):
                if k < NB:
                    gpsimd.wait_ge(d2g, k + 1)
                    if k >= 2:
                        gpsimd.wait_ge(v2p, k - 1)  # tts(k-2) freed rbuf slot
                    ins = gpsimd.tensor_tensor(
                        out=rbuf[:, k % 2, :],
                        in0=scanout[:, H : 1 + PW : H],
                        in1=scanout[:, 0 : PW : H],
                        op=A.subtract,
                    )
                    if has_b2:
                        ins = gpsimd.tensor_scalar(
                            rbuf[:, k % 2, :], rbuf[:, k % 2, :], k2[:],
                            None, A.add,
                        )
                    ins.then_inc(p2v, 1)
                if k >= 1:
                    kk = k - 1
                    pbi = kk % 2
                    gpsimd.wait_ge(v2p, kk + 1)
                    gpsimd.tensor_scalar(
                        sflag[:], u2b[:, pbi, :, :], 0.999, None, A.is_ge
                    )
                    gpsimd.tensor_tensor(
                        out=accf[:], in0=accf[:], in1=sflag[:], op=A.add
                    )
                    if kk >= WINBLK:
                        gpsimd.tensor_scalar(
                            swin[:], u2b[:, pbi, :, :], 1.0, None, A.is_ge
                        )
                        gpsimd.tensor_tensor(
                            out=accw[:], in0=accw[:], in1=swin[:], op=A.add
                        )
            # fold the 32 step-position accumulators down to [128, G]
            h = TCH
            while h > 1:
                h //= 2
                gpsimd.tensor_tensor(
                    out=accw[:, :, 0:h], in0=accw[:, :, 0:h],
                    in1=accw[:, :, h : 2 * h], op=A.add,
                )
                gpsimd.tensor_tensor(
                    out=accf[:, :, 0:h], in0=accf[:, :, 0:h],
                    in1=accf[:, :, h : 2 * h], op=A.add,
                )
            gpsimd.tensor_scalar(outb[:], accw[:, :, 0], 1.0, None, A.mult)
            gpsimd.tensor_scalar(
                flgb[:], accf[:, :, 0], 1.0, None, A.mult
            ).then_inc(g_done, 1)

    mybir.codegen_inst_isa_subclasses(nc)
    return nc


def _host_tiles(W1, b1, W2, b2):
    w1c = (W1 * 0.5).astype(np.float32)  # [H, I]
    wrhsb = np.zeros((128, 2 * TCH * HH), np.float32)
    for jc in range(2):
        for tp in range(TCH):
            for i in range(I):
                p = 64 * jc + I * tp + i
                for hf in range(2):
                    wrhsb[p, 512 * hf + HH * tp : 512 * hf + HH * (tp + 1)] = (
                        w1c[HH * hf : HH * (hf + 1), i]
                    )
    w2hb = np.tile((W2[0, :] * 0.5).astype(np.float32)[None, :], (128, G))
    k2b = np.full((128, 1), 0.5 * float(b2[0]), np.float32)
    b1hb = np.tile((b1 * 0.5).astype(np.float32)[None, :], (128, TCH))
    b1gb = np.tile((b1 * 0.5).astype(np.float32)[None, :], (128, G))
    return wrhsb, w2hb, k2b, b1hb, b1gb


def _host_xt(x_core, t_steps):
    """[B_CORE, T, I] -> transposed-tile layout [128, NCQ*G*128]."""
    NCQ = t_steps // (2 * TCH)
    xc = x_core.reshape(G, 128, NCQ, 2, TCH, I)    # [g, m, cq, jc, t', i]
    return np.ascontiguousarray(
        xc.transpose(3, 4, 5, 2, 0, 1).reshape(128, NCQ * G * 128)
    )


def _run(nc, shards, weights, want_flag):
    from concourse.bass_utils import run_bass_kernel_spmd

    wrhsb, w2hb, k2b, b1hb = weights
    in_maps = [
        {"xtb": s, "wrhsb": wrhsb, "w2hb": w2hb, "k2b": k2b, "b1hb": b1hb}
        for s in shards
    ]
    res = run_bass_kernel_spmd(nc, in_maps, list(range(N_CORES)))
    # out[p, g] holds batch row g*128 + p of the core's shard
    outs = [
        np.asarray(res.results[c]["out"]).T.reshape(B_CORE) for c in range(N_CORES)
    ]
    out = np.concatenate(outs).reshape(B, 1).astype(np.float32)
    if not want_flag:
        return out, None
    flag = sum(
        float(np.asarray(res.results[c]["flg"]).sum()) for c in range(N_CORES)
    )
    return out, flag


def kernel(x, W1, b1, W2, b2):
    W1 = np.asarray(W1); b1 = np.asarray(b1)
    W2 = np.asarray(W2); b2 = np.asarray(b2)
    has_b1 = bool(np.any(b1 != 0))
    has_b2 = bool(np.any(b2 != 0))

    kf = ("fast", T, has_b1, has_b2)
    if kf not in _cache:
        _cache[kf] = build_nc_fast(T, has_b1=has_b1, has_b2=has_b2)

    wrhsb, w2hb, k2b, b1hb, b1gb = _host_tiles(W1, b1, W2, b2)
    x = np.ascontiguousarray(np.asarray(x, np.float32))
    shards = [
        _host_xt(x[c * B_CORE : (c + 1) * B_CORE], T) for c in range(N_CORES)
    ]

    out, flag = _run(_cache[kf], shards, (wrhsb, w2hb, k2b, b1gb), True)
    if flag == 0.0:
        return out
    # no-reset layer-2 potential approached threshold somewhere: rerun with
    # the exact per-step layer-2 kernel
    ke = ("exact", T, has_b1, has_b2)
    if ke not in _cache:
        _cache[ke] = build_nc_exact(T, has_b1=has_b1, has_b2=has_b2)
    out, _ = _run(_cache[ke], shards, (wrhsb, w2hb, k2b, b1hb), False)
    return out

